# revision 41
# baseline (speedup 1.0000x reference)
"""Trainium2 8-core causal single-head attention.

Problem: x[4,4096,768] @ Wq/Wk/Wv[768,64] -> causal softmax attention -> out[4,4096,64].

Sharding: 8 cores = 4 batches x 2 query-interleave groups. Core c handles
batch b=c//2, parity h=c%2: local q-tile i (16 tiles of 128 rows) is the
global q-tile g=2i+h. Both cores of a batch compute full-context K/V
projections locally (no collectives).

Position remap (pure data, SPMD-uniform program): the host reorders the 32
kv tiles per core as [own q tiles 0..15 | other-parity tiles 0..15]. Local
q tile i then attends kv positions {0..i} (triangular mask on position i)
and {16..16+i} (parity mask on position 16+i: zeros for h=0, ones for h=1).
Masks are input data, so one compiled program serves both parities, and the
q projection reads plain contiguous chunks (no gather, no separate xq
stream: 6.3MB input instead of 9.4MB).

Host layout: xkv is chunk-major [8 chunks][128 rows][6 ec][512 cols], chunks
shipped in consume order Q0 Q1 N0 N1 Q2 Q3 N2 N3 (ship column s of kvT holds
position: s<8 -> s, 8<=s<16 -> 8+s, 16<=s<24 -> s-8, s>=24 -> s). Each chunk
is one contiguous 768KB DMA with 6KB/partition descriptors, alternating the
two hw-DGE queues (sync + scalar) in ship order. All constants ship as one
host-prepacked blob (single DMA).

On-chip: projections contract E on partitions; scores are computed as
S^T[kv_p, q_f] = kT.T @ qT so exp output PT directly feeds PV:
outT[d1, q] += v1[kv,65].T @ PT[kv, q], v1 = [v | ones] so row 64 accumulates
the softmax denominator (host divides + scatters). Exp runs on Scalar/ACT
(its only job - no DMA issue on that queue after the input phase); masks
multiply on DVE; each output 512-block drains as its accumulation closes,
the final block in two 256-col pieces to shorten the tail.
"""

import sys

sys.path.insert(0, "/opt/trn_rl_repo")

from contextlib import ExitStack

import numpy as np
import ml_dtypes

B, T, E, D = 4, 4096, 768, 64
P = 128
TQ = T // 2          # queries per core
NQT = TQ // P        # 16 local q tiles
NKV = T // P         # 32 kv tiles
EC = E // P          # 6 contraction chunks
NCH = 8              # xkv chunks of 512 cols (4 tiles each)
CHW = 512
BF16 = ml_dtypes.bfloat16
N_WARMUP = 14        # dummy matmuls covering the input-DMA landing window

# ship chunk s holds position-tiles POS_OF_CHUNK[s]
SHIP = [0, 1, 4, 5, 2, 3, 6, 7]          # ship slot -> q/n chunk id (Q0 Q1 N0 N1 Q2 Q3 N2 N3)


def shipcol(k):
    """kv position-pair k -> ship tile-column in kvT/v1."""
    if k < 8:
        return k
    if k < 16:
        return 8 + k          # positions 8..15 live in ship chunks 4,5
    if k < 24:
        return k - 8          # positions 16..23 live in ship chunks 2,3
    return k                  # positions 24..31 live in ship chunks 6,7

_CACHE = {}


def _build_bass():
    import concourse.bacc as bacc
    import concourse.mybir as mybir
    import concourse.tile as tile

    nc = bacc.Bacc("TRN2", target_bir_lowering=False)
    f32 = mybir.dt.float32
    bf16 = mybir.dt.bfloat16

    xkv_d = nc.dram_tensor("xkv", (NCH * P, EC * CHW), bf16, kind="ExternalInput")
    # constants host-prepacked in SBUF layout: one DMA, big descriptors.
    CW_Q, CW_KV, CW_M = EC * D, EC * 2 * D, P
    CTOT = CW_Q + CW_KV + 2 * CW_M + D
    const_d = nc.dram_tensor("consts", (P, CTOT), bf16, kind="ExternalInput")
    out_d = nc.dram_tensor("out", (D + 1, TQ), f32, kind="ExternalOutput")

    with ExitStack() as ctx:
        tc = ctx.enter_context(tile.TileContext(nc))
        const = ctx.enter_context(tc.tile_pool(name="const", bufs=1))
        xpool = ctx.enter_context(tc.tile_pool(name="x", bufs=1))
        spool = ctx.enter_context(tc.tile_pool(name="sb", bufs=1))
        ptpool = ctx.enter_context(tc.tile_pool(name="pt", bufs=4))
        obpool = ctx.enter_context(tc.tile_pool(name="ob", bufs=2))
        # PSUM: scores 2x[128,1024]f32 = 4 banks, proj 2x[128,512]f32-sized =
        # 2 banks, outp [65,1024]f32 = 2 banks -> 8 banks exactly. Separate
        # pools so a long-lived proj accumulator never steals a score buffer.
        psst = ctx.enter_context(tc.tile_pool(name="psst", bufs=2, space="PSUM"))
        pprj = ctx.enter_context(tc.tile_pool(name="pprj", bufs=2, space="PSUM"))
        pout = ctx.enter_context(tc.tile_pool(name="pout", bufs=1, space="PSUM"))

        # ---- PE warmup: keep TensorE busy through the input-DMA landing so
        # the HAM clock gate ramps and real matmuls start at 2.4 GHz ----
        scratch = const.tile([P, 512], bf16)
        nc.vector.memset(scratch[:], 1.0)

        def warm(n):
            for wi in range(n):
                pw = psst.tile([P, 1024], f32, tag="ss", name=f"warm{wi}")
                nc.tensor.matmul(
                    pw[:, 0:512], lhsT=scratch[:, 0:P], rhs=scratch[:],
                    start=True, stop=True,
                )
                if wi == n - 1:
                    nc.vector.tensor_copy(scratch[0:1, 0:1], pw[0:1, 0:1])

        warm(N_WARMUP)

        # ---- input DMA on the two hw-DGE queues in ship order ----
        const_t = const.tile([P, CTOT], bf16)
        nc.sync.dma_start(out=const_t[:], in_=const_d[:])
        wq_t = const_t[:, 0:CW_Q]
        wkv_t = const_t[:, CW_Q:CW_Q + CW_KV]
        mdiag_t = const_t[:, CW_Q + CW_KV:CW_Q + CW_KV + CW_M]
        mpar_t = const_t[:, CW_Q + CW_KV + CW_M:CW_Q + CW_KV + 2 * CW_M]
        ident_t = const_t[:, CW_Q + CW_KV + 2 * CW_M:CTOT]

        xkv_t = xpool.tile([P, NCH * EC * CHW], bf16)

        def dma_xkv(s, eng, c0=0, c1=EC * CHW):
            eng.dma_start(
                out=xkv_t[:, s * EC * CHW + c0: s * EC * CHW + c1],
                in_=xkv_d[s * P:(s + 1) * P, c0:c1],
            )

        for s in range(0, NCH, 2):
            dma_xkv(s, nc.scalar)
            dma_xkv(s + 1, nc.sync)

        qT_t = spool.tile([D, TQ], bf16)
        kvT_t = spool.tile([P, T], bf16)
        v1_t = spool.tile([P, NKV * (D + 1)], bf16)
        nc.vector.memset(v1_t[:], 1.0)

        def qt_step(jb, ec, box):
            # one ec-step of the q projection for qT cols [jb*512,(jb+1)*512)
            s = (0, 1, 4, 5)[jb]
            if ec == 0:
                box["ps"] = pprj.tile([P, 512], f32, tag="pj", name=f"psq{jb}")
            ps = box["ps"]
            nc.tensor.matmul(
                ps[0:D, :],
                lhsT=wq_t[:, ec * D:(ec + 1) * D],
                rhs=xkv_t[:, s * EC * CHW + ec * CHW:(s * EC + ec + 1) * CHW],
                start=(ec == 0),
                stop=(ec == EC - 1),
            )
            if ec == EC - 1:
                nc.vector.tensor_copy(qT_t[:, jb * 512:(jb + 1) * 512], ps[0:D, :])

        def kv_step(s, ec, box):
            # one ec-step of the k/v projection for ship columns s*512..
            if ec == 0:
                box["ps"] = pprj.tile([P, 512], f32, tag="pj", name=f"pskv{s}")
            ps = box["ps"]
            nc.tensor.matmul(
                ps[:, :],
                lhsT=wkv_t[:, ec * 2 * D:(ec + 1) * 2 * D],
                rhs=xkv_t[:, s * EC * CHW + ec * CHW:(s * EC + ec + 1) * CHW],
                start=(ec == 0),
                stop=(ec == EC - 1),
            )
            if ec == EC - 1:
                nc.vector.tensor_copy(kvT_t[:, s * 512:(s + 1) * 512], ps[:, :])

        def v_transpose(s):
            # transpose the 4 v-tiles of ship chunk s into v1
            pv = pprj.tile([P, 512], bf16, tag="pj", name=f"psv{s}")
            for m in range(4):
                k = 4 * s + m
                nc.tensor.transpose(
                    pv[:, m * D:(m + 1) * D],
                    in_=kvT_t[D:2 * D, k * P:(k + 1) * P],
                    identity=ident_t[D:2 * D, :],
                )
            nc.vector.tensor_copy(
                v1_t.rearrange("p (k e) -> p k e", e=D + 1)[:, 4 * s:4 * s + 4, 0:D],
                pv.rearrange("p (m e) -> p m e", e=D)[:, 0:4, :],
            )

        def qt_proj(jb):
            box = {}
            for ec in range(EC):
                qt_step(jb, ec, box)

        def kv_proj_mm(s):
            box = {}
            for ec in range(EC):
                kv_step(s, ec, box)

        # ---- fused attention pair stream with single-matmul proj fillers:
        # ACT stays gapless while the PE slots leftover projection work into
        # its per-pair slack instead of bursting it between groups ----
        fillers = []

        def add_kv(s):
            box = {}
            for ec in range(EC):
                fillers.append(lambda s=s, ec=ec, box=box: kv_step(s, ec, box))

        def add_qt(jb):
            box = {}
            for ec in range(EC):
                fillers.append(lambda jb=jb, ec=ec, box=box: qt_step(jb, ec, box))

        def add_tr(s):
            fillers.append(lambda s=s: v_transpose(s))

        add_kv(2)            # s2 = N0 (kv positions 16..19), need by slot 6
        add_tr(2)
        add_kv(3)            # s3 = N1, need by slot 8
        add_tr(3)
        add_qt(2)            # s4 = Q2, need by slot 12 (group 1 scores)
        add_qt(3)            # s5 = Q3, need by slot 12
        add_kv(4)            # s4 kv (positions 8..11), need by slot 28
        add_tr(4)
        add_kv(5)
        add_tr(5)
        add_kv(6)            # s6 = N2 (positions 24..27), need by slot 34
        add_tr(6)
        add_kv(7)
        add_tr(7)

        # slot stream: each slot is one PSUM score tile + ONE activation over
        # the packed segments [(k, cs, ce), ...]. Group 0's first four pairs
        # run their qt0-only left half first so exp starts ~4us earlier (qt1
        # waits on the second input chunk); narrow tail pairs share an
        # activation to amortize the ~300ns ACT fixed cost; group 1 leads
        # with full-width pairs whose data landed long ago.
        def seg(cq, k):
            return (k, max((k % 16) * P, cq * 1024), (cq + 1) * 1024)

        slots = [(0, [seg(0, k)]) for k in [0, 1, 2, 3]]
        slots += [(0, [seg(0, 4), seg(0, 5)]), (0, [seg(0, 6), seg(0, 7)])]
        slots += [(0, [seg(0, k)]) for k in [16, 17, 18, 19]]
        slots += [(0, [seg(0, 20), seg(0, 21)]), (0, [seg(0, 22), seg(0, 23)])]
        slots += [(1, [seg(1, k)]) for k in [0, 1, 2, 3, 4, 5, 6, 7,
                                             16, 17, 18, 19, 20, 21, 22, 23,
                                             8, 9, 10, 11]]
        slots += [(1, [seg(1, 12), seg(1, 13)]), (1, [seg(1, 14), seg(1, 15)])]
        slots += [(1, [seg(1, k)]) for k in [24, 25, 26, 27]]
        slots += [(1, [seg(1, 28), seg(1, 29)]), (1, [seg(1, 30), seg(1, 31)])]

        outp_tiles = {}
        pend = []

        def drain(outp, lo, c0, c1):
            ob = obpool.tile([D + 1, c1 - c0], f32)
            nc.vector.tensor_copy(ob[:], outp[:, c0 - lo: c1 - lo])
            nc.sync.dma_start(out=out_d[:, c0:c1], in_=ob[:])

        def flush_pv():
            cq, k, pt, po, cs, ce = pend.pop(0)
            lo = cq * 1024
            w = ce - cs
            outp = outp_tiles[cq]
            sc = shipcol(k)
            v1k = v1_t[:, sc * (D + 1):(sc + 1) * (D + 1)]
            # unmasked halves first: the mask multiply only touches
            # pt[:, po:po+128], so the later half issues without waiting on it
            for half in sorted(range(0, w, 512), reverse=True):
                hw = min(512, w - half)
                g512 = (cs + half) // 512
                nc.tensor.matmul(
                    outp[:, cs + half - lo: cs + half - lo + hw],
                    lhsT=v1k,
                    rhs=pt[:, po + half:po + half + hw],
                    start=(k == 0),
                    stop=(k == 16 + 4 * g512 + 3),
                )
            # drain blocks as their accumulation closes (the parity pair
            # 16+j of q tile j is always the last writer of its block);
            # final block goes in two 256-col pieces to shorten the tail
            if k == 16 + 4 * (2 * cq) + 3:
                drain(outp, lo, lo, lo + 512)
            elif k == 16 + 4 * (2 * cq + 1) + 3:
                if cq == 1:
                    drain(outp, lo, 1792, 2048)
                else:
                    drain(outp, lo, lo + 512, lo + 1024)
            elif cq == 1 and k == 30:
                drain(outp, lo, 1536, 1792)

        # front: projections feeding the first slots, in ship-arrival order
        qt_proj(0)            # s0
        kv_proj_mm(0)
        qt_proj(1)            # s1
        kv_proj_mm(1)
        v_transpose(0)
        v_transpose(1)

        nf = 0  # fillers consumed
        for idx, (cq, segs) in enumerate(slots):
            if cq not in outp_tiles:
                outp_tiles[cq] = pout.tile(
                    [D + 1, 1024], f32, tag="out", name=f"outp{cq}"
                )
            wtot = sum(ce - cs for _, cs, ce in segs)
            sst = psst.tile([P, 1024], f32, tag="ss", name=f"sst{idx}")
            po = 0
            offs = []
            for k, cs, ce in segs:
                offs.append(po)
                for half in range(0, ce - cs, 512):
                    hw = min(512, ce - cs - half)
                    nc.tensor.matmul(
                        sst[:, po + half:po + half + hw],
                        lhsT=kvT_t[0:D, shipcol(k) * P:(shipcol(k) + 1) * P],
                        rhs=qT_t[:, cs + half: cs + half + hw],
                        start=True,
                        stop=True,
                    )
                po += ce - cs
            pt = ptpool.tile([P, 1024], bf16)
            nc.scalar.activation(
                pt[:, 0:wtot], sst[:, 0:wtot],
                func=mybir.ActivationFunctionType.Exp, scale=0.125,
            )
            for (k, cs, ce), po in zip(segs, offs):
                if cs == (k % 16) * P:
                    m = mdiag_t if k < 16 else mpar_t
                    nc.vector.tensor_mul(
                        pt[:, po:po + P], pt[:, po:po + P], m[:]
                    )
                pend.append((cq, k, pt, po, cs, ce))
            # proj filler steps: front-loaded so group 1's prerequisites
            # (qt2/qt3) land by slot 12 and kv7 by the tail slots
            budget = 3 if idx < 4 else (2 if idx < 14 else 1)
            while budget > 0 and nf < len(fillers):
                fillers[nf]()
                nf += 1
                budget -= 1
            # scores of the next slot issue before the PV of this one: the PE
            # never sits through the Scalar-engine exp latency
            while len(pend) > max(2, len(segs)):
                flush_pv()
        while pend:
            flush_pv()
        assert nf == len(fillers), (nf, len(fillers))

    nc.compile()
    return nc


def _shard_inputs(x, Wq, Wk, Wv):
    x = np.asarray(x, np.float32)
    wq_p = np.asarray(Wq, np.float32).astype(BF16).reshape(EC, P, D).transpose(1, 0, 2).reshape(P, EC * D)
    wkv_p = np.concatenate(
        [np.asarray(Wk, np.float32), np.asarray(Wv, np.float32)], axis=1
    ).astype(BF16).reshape(EC, P, 2 * D).transpose(1, 0, 2).reshape(P, EC * 2 * D)
    ident = np.zeros((P, D), BF16)
    ident[D:2 * D, :] = np.eye(D, dtype=BF16)
    tri = (np.arange(P)[:, None] <= np.arange(P)[None, :]).astype(BF16)
    ones = np.ones((P, P), BF16)
    zeros = np.zeros((P, P), BF16)
    consts = {
        h: np.ascontiguousarray(np.concatenate(
            [wq_p, wkv_p, tri, zeros if h == 0 else ones, ident], axis=1))
        for h in (0, 1)
    }
    in_maps = []
    for c in range(8):
        b, h = c // 2, c % 2
        xT = np.ascontiguousarray(x[b].T).astype(BF16)      # [768, 4096]
        xt = xT.reshape(E, NKV, P)
        # position order: own q tiles (2i+h) first, then other parity
        pos = np.concatenate([2 * np.arange(NQT) + h, 2 * np.arange(NQT) + 1 - h])
        # ship order Q0 Q1 N0 N1 Q2 Q3 N2 N3 over position-chunks of 4 tiles
        tile_order = np.concatenate([pos[4 * s:4 * s + 4] for s in SHIP])
        xt = xt[:, tile_order, :]
        xc = np.ascontiguousarray(
            xt.reshape(EC, P, NCH, CHW).transpose(2, 1, 0, 3)
        ).reshape(NCH * P, EC * CHW)
        in_maps.append({"xkv": xc, "consts": consts[h]})
    return in_maps


def _unshard(results):
    out = np.zeros((B, T, D), np.float32)
    for c, om in enumerate(results):
        b, h = c // 2, c % 2
        o = np.asarray(om["out"], np.float32)               # [65, 2048]
        on = (o[:D] / o[D:D + 1]).T                         # [2048, 64]
        for i in range(NQT):
            out[b, (2 * i + h) * P:(2 * i + h + 1) * P] = on[i * P:(i + 1) * P]
    return out


def kernel(x, Wq, Wk, Wv):
    from concourse import bass_utils

    if "nc" not in _CACHE:
        _CACHE["nc"] = _build_bass()
    nc = _CACHE["nc"]
    in_maps = _shard_inputs(x, Wq, Wk, Wv)
    res = bass_utils.run_bass_kernel_spmd(nc, in_maps, core_ids=list(range(8)))
    _CACHE["last_result"] = res
    return _unshard(res.results)


# revision 42
# speedup vs baseline: 1.0301x; 1.0301x over previous
"""Trainium2 8-core causal single-head attention.

Problem: x[4,4096,768] @ Wq/Wk/Wv[768,64] -> causal softmax attention -> out[4,4096,64].

Sharding: 8 cores = 4 batches x 2 query-interleave groups. Core c handles
batch b=c//2, parity h=c%2: local q-tile i (16 tiles of 128 rows) is the
global q-tile g=2i+h. Both cores of a batch compute full-context K/V
projections locally (no collectives).

Position remap (pure data, SPMD-uniform program): the host reorders the 32
kv tiles per core as [own q tiles 0..15 | other-parity tiles 0..15]. Local
q tile i then attends kv positions {0..i} (triangular mask on position i)
and {16..16+i} (parity mask on position 16+i: zeros for h=0, ones for h=1).
Masks are input data, so one compiled program serves both parities, and the
q projection reads plain contiguous chunks (no gather, no separate xq
stream: 6.3MB input instead of 9.4MB).

Host layout: xkv is chunk-major [8 chunks][128 rows][6 ec][512 cols], chunks
shipped in consume order Q0 Q1 N0 N1 Q2 Q3 N2 N3 (ship column s of kvT holds
position: s<8 -> s, 8<=s<16 -> 8+s, 16<=s<24 -> s-8, s>=24 -> s). Each chunk
is one contiguous 768KB DMA with 6KB/partition descriptors, alternating the
two hw-DGE queues (sync + scalar) in ship order. All constants ship as one
host-prepacked blob (single DMA).

On-chip: projections contract E on partitions; scores are computed as
S^T[kv_p, q_f] = kT.T @ qT so exp output PT directly feeds PV:
outT[d1, q] += v1[kv,65].T @ PT[kv, q], v1 = [v | ones] so row 64 accumulates
the softmax denominator (host divides + scatters). Exp runs on Scalar/ACT
(its only job - no DMA issue on that queue after the input phase); masks
multiply on DVE; each output 512-block drains as its accumulation closes,
the final block in two 256-col pieces to shorten the tail.
"""

import sys

sys.path.insert(0, "/opt/trn_rl_repo")

from contextlib import ExitStack

import numpy as np
import ml_dtypes

B, T, E, D = 4, 4096, 768, 64
P = 128
TQ = T // 2          # queries per core
NQT = TQ // P        # 16 local q tiles
NKV = T // P         # 32 kv tiles
EC = E // P          # 6 contraction chunks
NCH = 8              # xkv chunks of 512 cols (4 tiles each)
CHW = 512
BF16 = ml_dtypes.bfloat16
N_WARMUP = 12        # dummy matmuls covering the input-DMA landing window

# ship chunk s holds position-tiles POS_OF_CHUNK[s]
SHIP = [0, 1, 4, 5, 2, 3, 6, 7]          # ship slot -> q/n chunk id (Q0 Q1 N0 N1 Q2 Q3 N2 N3)


def shipcol(k):
    """kv position-pair k -> ship tile-column in kvT/v1."""
    if k < 8:
        return k
    if k < 16:
        return 8 + k          # positions 8..15 live in ship chunks 4,5
    if k < 24:
        return k - 8          # positions 16..23 live in ship chunks 2,3
    return k                  # positions 24..31 live in ship chunks 6,7

_CACHE = {}


def _build_bass():
    import concourse.bacc as bacc
    import concourse.mybir as mybir
    import concourse.tile as tile

    nc = bacc.Bacc("TRN2", target_bir_lowering=False)
    f32 = mybir.dt.float32
    bf16 = mybir.dt.bfloat16

    xkv_d = nc.dram_tensor("xkv", (NCH * P, EC * CHW), bf16, kind="ExternalInput")
    # constants host-prepacked in SBUF layout: one DMA, big descriptors.
    CW_Q, CW_KV, CW_M = EC * D, EC * 2 * D, P
    CTOT = CW_Q + CW_KV + 2 * CW_M + D
    const_d = nc.dram_tensor("consts", (P, CTOT), bf16, kind="ExternalInput")
    out_d = nc.dram_tensor("out", (D + 1, TQ), f32, kind="ExternalOutput")

    with ExitStack() as ctx:
        tc = ctx.enter_context(tile.TileContext(nc))
        const = ctx.enter_context(tc.tile_pool(name="const", bufs=1))
        xpool = ctx.enter_context(tc.tile_pool(name="x", bufs=1))
        spool = ctx.enter_context(tc.tile_pool(name="sb", bufs=1))
        ptpool = ctx.enter_context(tc.tile_pool(name="pt", bufs=4))
        obpool = ctx.enter_context(tc.tile_pool(name="ob", bufs=2))
        # PSUM: scores 2x[128,1024]f32 = 4 banks, proj 2x[128,512]f32-sized =
        # 2 banks, outp [65,1024]f32 = 2 banks -> 8 banks exactly. Separate
        # pools so a long-lived proj accumulator never steals a score buffer.
        psst = ctx.enter_context(tc.tile_pool(name="psst", bufs=2, space="PSUM"))
        pprj = ctx.enter_context(tc.tile_pool(name="pprj", bufs=2, space="PSUM"))
        pout = ctx.enter_context(tc.tile_pool(name="pout", bufs=1, space="PSUM"))

        # ---- PE warmup: keep TensorE busy through the input-DMA landing so
        # the HAM clock gate ramps and real matmuls start at 2.4 GHz ----
        scratch = const.tile([P, 512], bf16)
        nc.vector.memset(scratch[:], 1.0)

        def warm(n):
            for wi in range(n):
                pw = psst.tile([P, 1024], f32, tag="ss", name=f"warm{wi}")
                nc.tensor.matmul(
                    pw[:, 0:512], lhsT=scratch[:, 0:P], rhs=scratch[:],
                    start=True, stop=True,
                )
                if wi == n - 1:
                    nc.vector.tensor_copy(scratch[0:1, 0:1], pw[0:1, 0:1])

        warm(N_WARMUP)

        # ---- input DMA on the two hw-DGE queues in ship order ----
        const_t = const.tile([P, CTOT], bf16)
        nc.sync.dma_start(out=const_t[:], in_=const_d[:])
        wq_t = const_t[:, 0:CW_Q]
        wkv_t = const_t[:, CW_Q:CW_Q + CW_KV]
        mdiag_t = const_t[:, CW_Q + CW_KV:CW_Q + CW_KV + CW_M]
        mpar_t = const_t[:, CW_Q + CW_KV + CW_M:CW_Q + CW_KV + 2 * CW_M]
        ident_t = const_t[:, CW_Q + CW_KV + 2 * CW_M:CTOT]

        xkv_t = xpool.tile([P, NCH * EC * CHW], bf16)

        def dma_xkv(s, eng, c0=0, c1=EC * CHW):
            eng.dma_start(
                out=xkv_t[:, s * EC * CHW + c0: s * EC * CHW + c1],
                in_=xkv_d[s * P:(s + 1) * P, c0:c1],
            )

        for s in range(0, NCH, 2):
            dma_xkv(s, nc.scalar)
            dma_xkv(s + 1, nc.sync)

        qT_t = spool.tile([D, TQ], bf16)
        kvT_t = spool.tile([P, T], bf16)
        v1_t = spool.tile([P, NKV * (D + 1)], bf16)
        nc.vector.memset(v1_t[:], 1.0)

        def qt_step(jb, ec, box):
            # one ec-step of the q projection for qT cols [jb*512,(jb+1)*512)
            s = (0, 1, 4, 5)[jb]
            if ec == 0:
                box["ps"] = pprj.tile([P, 512], f32, tag="pj", name=f"psq{jb}")
            ps = box["ps"]
            nc.tensor.matmul(
                ps[0:D, :],
                lhsT=wq_t[:, ec * D:(ec + 1) * D],
                rhs=xkv_t[:, s * EC * CHW + ec * CHW:(s * EC + ec + 1) * CHW],
                start=(ec == 0),
                stop=(ec == EC - 1),
            )
            if ec == EC - 1:
                nc.vector.tensor_copy(qT_t[:, jb * 512:(jb + 1) * 512], ps[0:D, :])

        def kv_step(s, ec, box):
            # one ec-step of the k/v projection for ship columns s*512..
            if ec == 0:
                box["ps"] = pprj.tile([P, 512], f32, tag="pj", name=f"pskv{s}")
            ps = box["ps"]
            nc.tensor.matmul(
                ps[:, :],
                lhsT=wkv_t[:, ec * 2 * D:(ec + 1) * 2 * D],
                rhs=xkv_t[:, s * EC * CHW + ec * CHW:(s * EC + ec + 1) * CHW],
                start=(ec == 0),
                stop=(ec == EC - 1),
            )
            if ec == EC - 1:
                nc.vector.tensor_copy(kvT_t[:, s * 512:(s + 1) * 512], ps[:, :])

        def v_transpose(s):
            # transpose the 4 v-tiles of ship chunk s into v1
            pv = pprj.tile([P, 512], bf16, tag="pj", name=f"psv{s}")
            for m in range(4):
                k = 4 * s + m
                nc.tensor.transpose(
                    pv[:, m * D:(m + 1) * D],
                    in_=kvT_t[D:2 * D, k * P:(k + 1) * P],
                    identity=ident_t[D:2 * D, :],
                )
            nc.vector.tensor_copy(
                v1_t.rearrange("p (k e) -> p k e", e=D + 1)[:, 4 * s:4 * s + 4, 0:D],
                pv.rearrange("p (m e) -> p m e", e=D)[:, 0:4, :],
            )

        def qt_proj(jb):
            box = {}
            for ec in range(EC):
                qt_step(jb, ec, box)

        def kv_proj_mm(s):
            box = {}
            for ec in range(EC):
                kv_step(s, ec, box)

        # ---- fused attention pair stream with single-matmul proj fillers:
        # ACT stays gapless while the PE slots leftover projection work into
        # its per-pair slack instead of bursting it between groups ----
        fillers = []

        def add_kv(s):
            box = {}
            for ec in range(EC):
                fillers.append(lambda s=s, ec=ec, box=box: kv_step(s, ec, box))

        def add_qt(jb):
            box = {}
            for ec in range(EC):
                fillers.append(lambda jb=jb, ec=ec, box=box: qt_step(jb, ec, box))

        def add_tr(s):
            fillers.append(lambda s=s: v_transpose(s))

        add_kv(2)            # s2 = N0 (kv positions 16..19), need by slot 6
        add_tr(2)
        add_kv(3)            # s3 = N1, need by slot 8
        add_tr(3)
        add_qt(2)            # s4 = Q2, need by slot 12 (group 1 scores)
        add_qt(3)            # s5 = Q3, need by slot 12
        add_kv(4)            # s4 kv (positions 8..11), need by slot 28
        add_tr(4)
        add_kv(5)
        add_tr(5)
        add_kv(6)            # s6 = N2 (positions 24..27), need by slot 34
        add_tr(6)
        add_kv(7)
        add_tr(7)

        # slot stream: each slot is one PSUM score tile + ONE activation over
        # the packed segments [(k, cs, ce), ...]. Group 0's first four pairs
        # run their qt0-only left half first so exp starts ~4us earlier (qt1
        # waits on the second input chunk); narrow tail pairs share an
        # activation to amortize the ~300ns ACT fixed cost; group 1 leads
        # with full-width pairs whose data landed long ago.
        def seg(cq, k):
            return (k, max((k % 16) * P, cq * 1024), (cq + 1) * 1024)

        slots = [(0, [seg(0, k)]) for k in [0, 1, 2, 3]]
        slots += [(0, [seg(0, 4), seg(0, 5)]), (0, [seg(0, 6), seg(0, 7)])]
        slots += [(0, [seg(0, k)]) for k in [16, 17, 18, 19]]
        slots += [(0, [seg(0, 20), seg(0, 21)]), (0, [seg(0, 22), seg(0, 23)])]
        slots += [(1, [seg(1, k)]) for k in [0, 1, 2, 3, 4, 5, 6, 7,
                                             16, 17, 18, 19, 20, 21, 22, 23,
                                             8, 9, 10, 11]]
        slots += [(1, [seg(1, 12), seg(1, 13)]), (1, [seg(1, 14), seg(1, 15)])]
        slots += [(1, [seg(1, k)]) for k in [24, 25, 26, 27]]
        slots += [(1, [seg(1, 28), seg(1, 29)]), (1, [seg(1, 30), seg(1, 31)])]

        outp_tiles = {}
        pend = []

        def drain(outp, lo, c0, c1):
            ob = obpool.tile([D + 1, c1 - c0], f32)
            nc.vector.tensor_copy(ob[:], outp[:, c0 - lo: c1 - lo])
            nc.sync.dma_start(out=out_d[:, c0:c1], in_=ob[:])

        def flush_pv():
            cq, k, pt, po, cs, ce = pend.pop(0)
            lo = cq * 1024
            w = ce - cs
            outp = outp_tiles[cq]
            sc = shipcol(k)
            v1k = v1_t[:, sc * (D + 1):(sc + 1) * (D + 1)]
            # unmasked halves first: the mask multiply only touches
            # pt[:, po:po+128], so the later half issues without waiting on it
            for half in sorted(range(0, w, 512), reverse=True):
                hw = min(512, w - half)
                g512 = (cs + half) // 512
                nc.tensor.matmul(
                    outp[:, cs + half - lo: cs + half - lo + hw],
                    lhsT=v1k,
                    rhs=pt[:, po + half:po + half + hw],
                    start=(k == 0),
                    stop=(k == 16 + 4 * g512 + 3),
                )
            # drain blocks as their accumulation closes (the parity pair
            # 16+j of q tile j is always the last writer of its block);
            # final block goes in two 256-col pieces to shorten the tail
            if k == 16 + 4 * (2 * cq) + 3:
                drain(outp, lo, lo, lo + 512)
            elif k == 16 + 4 * (2 * cq + 1) + 3:
                if cq == 1:
                    drain(outp, lo, 1792, 2048)
                else:
                    drain(outp, lo, lo + 512, lo + 1024)
            elif cq == 1 and k == 30:
                drain(outp, lo, 1536, 1792)

        # front: projections feeding the first slots, in ship-arrival order
        qt_proj(0)            # s0
        kv_proj_mm(0)
        qt_proj(1)            # s1
        kv_proj_mm(1)
        v_transpose(0)
        v_transpose(1)

        nf = 0  # fillers consumed
        for idx, (cq, segs) in enumerate(slots):
            if cq not in outp_tiles:
                outp_tiles[cq] = pout.tile(
                    [D + 1, 1024], f32, tag="out", name=f"outp{cq}"
                )
            wtot = sum(ce - cs for _, cs, ce in segs)
            sst = psst.tile([P, 1024], f32, tag="ss", name=f"sst{idx}")
            po = 0
            offs = []
            for k, cs, ce in segs:
                offs.append(po)
                for half in range(0, ce - cs, 512):
                    hw = min(512, ce - cs - half)
                    nc.tensor.matmul(
                        sst[:, po + half:po + half + hw],
                        lhsT=kvT_t[0:D, shipcol(k) * P:(shipcol(k) + 1) * P],
                        rhs=qT_t[:, cs + half: cs + half + hw],
                        start=True,
                        stop=True,
                    )
                po += ce - cs
            pt = ptpool.tile([P, 1024], bf16)
            nc.scalar.activation(
                pt[:, 0:wtot], sst[:, 0:wtot],
                func=mybir.ActivationFunctionType.Exp, scale=0.125,
            )
            for (k, cs, ce), po in zip(segs, offs):
                if cs == (k % 16) * P:
                    m = mdiag_t if k < 16 else mpar_t
                    nc.vector.tensor_mul(
                        pt[:, po:po + P], pt[:, po:po + P], m[:]
                    )
                pend.append((cq, k, pt, po, cs, ce))
            # proj filler steps: front-loaded so group 1's prerequisites
            # (qt2/qt3) land by slot 12 and kv7 by the tail slots
            budget = 3 if idx < 4 else (2 if idx < 14 else 1)
            while budget > 0 and nf < len(fillers):
                fillers[nf]()
                nf += 1
                budget -= 1
            # scores of the next slot issue before the PV of this one: the PE
            # never sits through the Scalar-engine exp latency
            while len(pend) > max(2, len(segs)):
                flush_pv()
        while pend:
            flush_pv()
        assert nf == len(fillers), (nf, len(fillers))

    nc.compile()
    return nc


def _shard_inputs(x, Wq, Wk, Wv):
    x = np.asarray(x, np.float32)
    wq_p = np.asarray(Wq, np.float32).astype(BF16).reshape(EC, P, D).transpose(1, 0, 2).reshape(P, EC * D)
    wkv_p = np.concatenate(
        [np.asarray(Wk, np.float32), np.asarray(Wv, np.float32)], axis=1
    ).astype(BF16).reshape(EC, P, 2 * D).transpose(1, 0, 2).reshape(P, EC * 2 * D)
    ident = np.zeros((P, D), BF16)
    ident[D:2 * D, :] = np.eye(D, dtype=BF16)
    tri = (np.arange(P)[:, None] <= np.arange(P)[None, :]).astype(BF16)
    ones = np.ones((P, P), BF16)
    zeros = np.zeros((P, P), BF16)
    consts = {
        h: np.ascontiguousarray(np.concatenate(
            [wq_p, wkv_p, tri, zeros if h == 0 else ones, ident], axis=1))
        for h in (0, 1)
    }
    in_maps = []
    for c in range(8):
        b, h = c // 2, c % 2
        xT = np.ascontiguousarray(x[b].T).astype(BF16)      # [768, 4096]
        xt = xT.reshape(E, NKV, P)
        # position order: own q tiles (2i+h) first, then other parity
        pos = np.concatenate([2 * np.arange(NQT) + h, 2 * np.arange(NQT) + 1 - h])
        # ship order Q0 Q1 N0 N1 Q2 Q3 N2 N3 over position-chunks of 4 tiles
        tile_order = np.concatenate([pos[4 * s:4 * s + 4] for s in SHIP])
        xt = xt[:, tile_order, :]
        xc = np.ascontiguousarray(
            xt.reshape(EC, P, NCH, CHW).transpose(2, 1, 0, 3)
        ).reshape(NCH * P, EC * CHW)
        in_maps.append({"xkv": xc, "consts": consts[h]})
    return in_maps


def _unshard(results):
    out = np.zeros((B, T, D), np.float32)
    for c, om in enumerate(results):
        b, h = c // 2, c % 2
        o = np.asarray(om["out"], np.float32)               # [65, 2048]
        on = (o[:D] / o[D:D + 1]).T                         # [2048, 64]
        for i in range(NQT):
            out[b, (2 * i + h) * P:(2 * i + h + 1) * P] = on[i * P:(i + 1) * P]
    return out


def kernel(x, Wq, Wk, Wv):
    from concourse import bass_utils

    if "nc" not in _CACHE:
        _CACHE["nc"] = _build_bass()
    nc = _CACHE["nc"]
    in_maps = _shard_inputs(x, Wq, Wk, Wv)
    res = bass_utils.run_bass_kernel_spmd(nc, in_maps, core_ids=list(range(8)))
    _CACHE["last_result"] = res
    return _unshard(res.results)


# revision 43
# speedup vs baseline: 1.0302x; 1.0001x over previous
"""Trainium2 8-core causal single-head attention.

Problem: x[4,4096,768] @ Wq/Wk/Wv[768,64] -> causal softmax attention -> out[4,4096,64].

Sharding: 8 cores = 4 batches x 2 query-interleave groups. Core c handles
batch b=c//2, parity h=c%2: local q-tile i (16 tiles of 128 rows) is the
global q-tile g=2i+h. Both cores of a batch compute full-context K/V
projections locally (no collectives).

Position remap (pure data, SPMD-uniform program): the host reorders the 32
kv tiles per core as [own q tiles 0..15 | other-parity tiles 0..15]. Local
q tile i then attends kv positions {0..i} (triangular mask on position i)
and {16..16+i} (parity mask on position 16+i: zeros for h=0, ones for h=1).
Masks are input data, so one compiled program serves both parities, and the
q projection reads plain contiguous chunks (no gather, no separate xq
stream: 6.3MB input instead of 9.4MB).

Host layout: xkv is chunk-major [8 chunks][128 rows][6 ec][512 cols], chunks
shipped in consume order Q0 Q1 N0 N1 Q2 Q3 N2 N3 (ship column s of kvT holds
position: s<8 -> s, 8<=s<16 -> 8+s, 16<=s<24 -> s-8, s>=24 -> s). Each chunk
is one contiguous 768KB DMA with 6KB/partition descriptors, alternating the
two hw-DGE queues (sync + scalar) in ship order. All constants ship as one
host-prepacked blob (single DMA).

On-chip: projections contract E on partitions; scores are computed as
S^T[kv_p, q_f] = kT.T @ qT so exp output PT directly feeds PV:
outT[d1, q] += v1[kv,65].T @ PT[kv, q], v1 = [v | ones] so row 64 accumulates
the softmax denominator (host divides + scatters). Exp runs on Scalar/ACT
(its only job - no DMA issue on that queue after the input phase); masks
multiply on DVE; each output 512-block drains as its accumulation closes,
the final block in two 256-col pieces to shorten the tail.
"""

import sys

sys.path.insert(0, "/opt/trn_rl_repo")

from contextlib import ExitStack

import numpy as np
import ml_dtypes

B, T, E, D = 4, 4096, 768, 64
P = 128
TQ = T // 2          # queries per core
NQT = TQ // P        # 16 local q tiles
NKV = T // P         # 32 kv tiles
EC = E // P          # 6 contraction chunks
NCH = 8              # xkv chunks of 512 cols (4 tiles each)
CHW = 512
BF16 = ml_dtypes.bfloat16
N_WARMUP = 12        # dummy matmuls covering the input-DMA landing window

# ship chunk s holds position-tiles POS_OF_CHUNK[s]
SHIP = [0, 1, 4, 5, 2, 3, 6, 7]          # ship slot -> q/n chunk id (Q0 Q1 N0 N1 Q2 Q3 N2 N3)


def shipcol(k):
    """kv position-pair k -> ship tile-column in kvT/v1."""
    if k < 8:
        return k
    if k < 16:
        return 8 + k          # positions 8..15 live in ship chunks 4,5
    if k < 24:
        return k - 8          # positions 16..23 live in ship chunks 2,3
    return k                  # positions 24..31 live in ship chunks 6,7

_CACHE = {}


def _build_bass():
    import concourse.bacc as bacc
    import concourse.mybir as mybir
    import concourse.tile as tile

    nc = bacc.Bacc("TRN2", target_bir_lowering=False)
    f32 = mybir.dt.float32
    bf16 = mybir.dt.bfloat16

    xkv_d = nc.dram_tensor("xkv", (NCH * P, EC * CHW), bf16, kind="ExternalInput")
    # constants host-prepacked in SBUF layout: one DMA, big descriptors.
    CW_Q, CW_KV, CW_M = EC * D, EC * 2 * D, P
    CTOT = CW_Q + CW_KV + 2 * CW_M + D
    const_d = nc.dram_tensor("consts", (P, CTOT), bf16, kind="ExternalInput")
    out_d = nc.dram_tensor("out", (D + 1, TQ), f32, kind="ExternalOutput")

    with ExitStack() as ctx:
        tc = ctx.enter_context(tile.TileContext(nc))
        const = ctx.enter_context(tc.tile_pool(name="const", bufs=1))
        xpool = ctx.enter_context(tc.tile_pool(name="x", bufs=1))
        spool = ctx.enter_context(tc.tile_pool(name="sb", bufs=1))
        ptpool = ctx.enter_context(tc.tile_pool(name="pt", bufs=5))
        obpool = ctx.enter_context(tc.tile_pool(name="ob", bufs=2))
        # PSUM: scores 2x[128,1024]f32 = 4 banks, proj 2x[128,512]f32-sized =
        # 2 banks, outp [65,1024]f32 = 2 banks -> 8 banks exactly. Separate
        # pools so a long-lived proj accumulator never steals a score buffer.
        psst = ctx.enter_context(tc.tile_pool(name="psst", bufs=2, space="PSUM"))
        pprj = ctx.enter_context(tc.tile_pool(name="pprj", bufs=2, space="PSUM"))
        pout = ctx.enter_context(tc.tile_pool(name="pout", bufs=1, space="PSUM"))

        # ---- PE warmup: keep TensorE busy through the input-DMA landing so
        # the HAM clock gate ramps and real matmuls start at 2.4 GHz ----
        scratch = const.tile([P, 512], bf16)
        nc.vector.memset(scratch[:], 1.0)

        def warm(n):
            for wi in range(n):
                pw = psst.tile([P, 1024], f32, tag="ss", name=f"warm{wi}")
                nc.tensor.matmul(
                    pw[:, 0:512], lhsT=scratch[:, 0:P], rhs=scratch[:],
                    start=True, stop=True,
                )
                if wi == n - 1:
                    nc.vector.tensor_copy(scratch[0:1, 0:1], pw[0:1, 0:1])

        warm(N_WARMUP)

        # ---- input DMA on the two hw-DGE queues in ship order ----
        const_t = const.tile([P, CTOT], bf16)
        nc.sync.dma_start(out=const_t[:], in_=const_d[:])
        wq_t = const_t[:, 0:CW_Q]
        wkv_t = const_t[:, CW_Q:CW_Q + CW_KV]
        mdiag_t = const_t[:, CW_Q + CW_KV:CW_Q + CW_KV + CW_M]
        mpar_t = const_t[:, CW_Q + CW_KV + CW_M:CW_Q + CW_KV + 2 * CW_M]
        ident_t = const_t[:, CW_Q + CW_KV + 2 * CW_M:CTOT]

        xkv_t = xpool.tile([P, NCH * EC * CHW], bf16)

        def dma_xkv(s, eng, c0=0, c1=EC * CHW):
            eng.dma_start(
                out=xkv_t[:, s * EC * CHW + c0: s * EC * CHW + c1],
                in_=xkv_d[s * P:(s + 1) * P, c0:c1],
            )

        for s in range(0, NCH, 2):
            dma_xkv(s, nc.scalar)
            dma_xkv(s + 1, nc.sync)

        qT_t = spool.tile([D, TQ], bf16)
        kvT_t = spool.tile([P, T], bf16)
        v1_t = spool.tile([P, NKV * (D + 1)], bf16)
        nc.vector.memset(v1_t[:], 1.0)

        def qt_step(jb, ec, box):
            # one ec-step of the q projection for qT cols [jb*512,(jb+1)*512)
            s = (0, 1, 4, 5)[jb]
            if ec == 0:
                box["ps"] = pprj.tile([P, 512], f32, tag="pj", name=f"psq{jb}")
            ps = box["ps"]
            nc.tensor.matmul(
                ps[0:D, :],
                lhsT=wq_t[:, ec * D:(ec + 1) * D],
                rhs=xkv_t[:, s * EC * CHW + ec * CHW:(s * EC + ec + 1) * CHW],
                start=(ec == 0),
                stop=(ec == EC - 1),
            )
            if ec == EC - 1:
                nc.vector.tensor_copy(qT_t[:, jb * 512:(jb + 1) * 512], ps[0:D, :])

        def kv_step(s, ec, box):
            # one ec-step of the k/v projection for ship columns s*512..
            if ec == 0:
                box["ps"] = pprj.tile([P, 512], f32, tag="pj", name=f"pskv{s}")
            ps = box["ps"]
            nc.tensor.matmul(
                ps[:, :],
                lhsT=wkv_t[:, ec * 2 * D:(ec + 1) * 2 * D],
                rhs=xkv_t[:, s * EC * CHW + ec * CHW:(s * EC + ec + 1) * CHW],
                start=(ec == 0),
                stop=(ec == EC - 1),
            )
            if ec == EC - 1:
                nc.vector.tensor_copy(kvT_t[:, s * 512:(s + 1) * 512], ps[:, :])

        def v_transpose(s):
            # transpose the 4 v-tiles of ship chunk s into v1
            pv = pprj.tile([P, 512], bf16, tag="pj", name=f"psv{s}")
            for m in range(4):
                k = 4 * s + m
                nc.tensor.transpose(
                    pv[:, m * D:(m + 1) * D],
                    in_=kvT_t[D:2 * D, k * P:(k + 1) * P],
                    identity=ident_t[D:2 * D, :],
                )
            nc.vector.tensor_copy(
                v1_t.rearrange("p (k e) -> p k e", e=D + 1)[:, 4 * s:4 * s + 4, 0:D],
                pv.rearrange("p (m e) -> p m e", e=D)[:, 0:4, :],
            )

        def qt_proj(jb):
            box = {}
            for ec in range(EC):
                qt_step(jb, ec, box)

        def kv_proj_mm(s):
            box = {}
            for ec in range(EC):
                kv_step(s, ec, box)

        # ---- fused attention pair stream with single-matmul proj fillers:
        # ACT stays gapless while the PE slots leftover projection work into
        # its per-pair slack instead of bursting it between groups ----
        fillers = []

        def add_kv(s):
            box = {}
            for ec in range(EC):
                fillers.append(lambda s=s, ec=ec, box=box: kv_step(s, ec, box))

        def add_qt(jb):
            box = {}
            for ec in range(EC):
                fillers.append(lambda jb=jb, ec=ec, box=box: qt_step(jb, ec, box))

        def add_tr(s):
            fillers.append(lambda s=s: v_transpose(s))

        add_kv(2)            # s2 = N0 (kv positions 16..19), need by slot 6
        add_tr(2)
        add_kv(3)            # s3 = N1, need by slot 8
        add_tr(3)
        add_qt(2)            # s4 = Q2, need by slot 12 (group 1 scores)
        add_qt(3)            # s5 = Q3, need by slot 12
        add_kv(4)            # s4 kv (positions 8..11), need by slot 28
        add_tr(4)
        add_kv(5)
        add_tr(5)
        add_kv(6)            # s6 = N2 (positions 24..27), need by slot 34
        add_tr(6)
        add_kv(7)
        add_tr(7)

        # slot stream: each slot is one PSUM score tile + ONE activation over
        # the packed segments [(k, cs, ce), ...]. Group 0's first four pairs
        # run their qt0-only left half first so exp starts ~4us earlier (qt1
        # waits on the second input chunk); narrow tail pairs share an
        # activation to amortize the ~300ns ACT fixed cost; group 1 leads
        # with full-width pairs whose data landed long ago.
        def seg(cq, k):
            return (k, max((k % 16) * P, cq * 1024), (cq + 1) * 1024)

        slots = [(0, [seg(0, k)]) for k in [0, 1, 2, 3]]
        slots += [(0, [seg(0, 4), seg(0, 5)]), (0, [seg(0, 6), seg(0, 7)])]
        slots += [(0, [seg(0, k)]) for k in [16, 17, 18, 19]]
        slots += [(0, [seg(0, 20), seg(0, 21)]), (0, [seg(0, 22), seg(0, 23)])]
        slots += [(1, [seg(1, k)]) for k in [0, 1, 2, 3, 4, 5, 6, 7,
                                             16, 17, 18, 19, 20, 21, 22, 23,
                                             8, 9, 10, 11]]
        slots += [(1, [seg(1, 12), seg(1, 13)]), (1, [seg(1, 14), seg(1, 15)])]
        slots += [(1, [seg(1, k)]) for k in [24, 25, 26, 27]]
        slots += [(1, [seg(1, 28), seg(1, 29)]), (1, [seg(1, 30), seg(1, 31)])]

        outp_tiles = {}
        pend = []

        def drain(outp, lo, c0, c1):
            ob = obpool.tile([D + 1, c1 - c0], f32)
            nc.vector.tensor_copy(ob[:], outp[:, c0 - lo: c1 - lo])
            nc.sync.dma_start(out=out_d[:, c0:c1], in_=ob[:])

        def flush_pv():
            cq, k, pt, po, cs, ce = pend.pop(0)
            lo = cq * 1024
            w = ce - cs
            outp = outp_tiles[cq]
            sc = shipcol(k)
            v1k = v1_t[:, sc * (D + 1):(sc + 1) * (D + 1)]
            # unmasked halves first: the mask multiply only touches
            # pt[:, po:po+128], so the later half issues without waiting on it
            for half in sorted(range(0, w, 512), reverse=True):
                hw = min(512, w - half)
                g512 = (cs + half) // 512
                nc.tensor.matmul(
                    outp[:, cs + half - lo: cs + half - lo + hw],
                    lhsT=v1k,
                    rhs=pt[:, po + half:po + half + hw],
                    start=(k == 0),
                    stop=(k == 16 + 4 * g512 + 3),
                )
            # drain blocks as their accumulation closes (the parity pair
            # 16+j of q tile j is always the last writer of its block);
            # final block goes in two 256-col pieces to shorten the tail
            if k == 16 + 4 * (2 * cq) + 3:
                drain(outp, lo, lo, lo + 512)
            elif k == 16 + 4 * (2 * cq + 1) + 3:
                if cq == 1:
                    drain(outp, lo, 1792, 2048)
                else:
                    drain(outp, lo, lo + 512, lo + 1024)
            elif cq == 1 and k == 30:
                drain(outp, lo, 1536, 1792)

        # front: projections feeding the first slots, in ship-arrival order
        qt_proj(0)            # s0
        kv_proj_mm(0)
        qt_proj(1)            # s1
        kv_proj_mm(1)
        v_transpose(0)
        v_transpose(1)

        nf = 0  # fillers consumed
        for idx, (cq, segs) in enumerate(slots):
            if cq not in outp_tiles:
                outp_tiles[cq] = pout.tile(
                    [D + 1, 1024], f32, tag="out", name=f"outp{cq}"
                )
            wtot = sum(ce - cs for _, cs, ce in segs)
            sst = psst.tile([P, 1024], f32, tag="ss", name=f"sst{idx}")
            po = 0
            offs = []
            for k, cs, ce in segs:
                offs.append(po)
                for half in range(0, ce - cs, 512):
                    hw = min(512, ce - cs - half)
                    nc.tensor.matmul(
                        sst[:, po + half:po + half + hw],
                        lhsT=kvT_t[0:D, shipcol(k) * P:(shipcol(k) + 1) * P],
                        rhs=qT_t[:, cs + half: cs + half + hw],
                        start=True,
                        stop=True,
                    )
                po += ce - cs
            pt = ptpool.tile([P, 1024], bf16)
            nc.scalar.activation(
                pt[:, 0:wtot], sst[:, 0:wtot],
                func=mybir.ActivationFunctionType.Exp, scale=0.125,
            )
            for (k, cs, ce), po in zip(segs, offs):
                if cs == (k % 16) * P:
                    m = mdiag_t if k < 16 else mpar_t
                    nc.vector.tensor_mul(
                        pt[:, po:po + P], pt[:, po:po + P], m[:]
                    )
                pend.append((cq, k, pt, po, cs, ce))
            # proj filler steps: front-loaded so group 1's prerequisites
            # (qt2/qt3) land by slot 12 and kv7 by the tail slots
            budget = 3 if idx < 4 else (2 if idx < 14 else 1)
            while budget > 0 and nf < len(fillers):
                fillers[nf]()
                nf += 1
                budget -= 1
            # scores of the next slot issue before the PV of this one: the PE
            # never sits through the Scalar-engine exp latency
            while len(pend) > max(3, len(segs)):
                flush_pv()
        while pend:
            flush_pv()
        assert nf == len(fillers), (nf, len(fillers))

    nc.compile()
    return nc


def _shard_inputs(x, Wq, Wk, Wv):
    x = np.asarray(x, np.float32)
    wq_p = np.asarray(Wq, np.float32).astype(BF16).reshape(EC, P, D).transpose(1, 0, 2).reshape(P, EC * D)
    wkv_p = np.concatenate(
        [np.asarray(Wk, np.float32), np.asarray(Wv, np.float32)], axis=1
    ).astype(BF16).reshape(EC, P, 2 * D).transpose(1, 0, 2).reshape(P, EC * 2 * D)
    ident = np.zeros((P, D), BF16)
    ident[D:2 * D, :] = np.eye(D, dtype=BF16)
    tri = (np.arange(P)[:, None] <= np.arange(P)[None, :]).astype(BF16)
    ones = np.ones((P, P), BF16)
    zeros = np.zeros((P, P), BF16)
    consts = {
        h: np.ascontiguousarray(np.concatenate(
            [wq_p, wkv_p, tri, zeros if h == 0 else ones, ident], axis=1))
        for h in (0, 1)
    }
    in_maps = []
    for c in range(8):
        b, h = c // 2, c % 2
        xT = np.ascontiguousarray(x[b].T).astype(BF16)      # [768, 4096]
        xt = xT.reshape(E, NKV, P)
        # position order: own q tiles (2i+h) first, then other parity
        pos = np.concatenate([2 * np.arange(NQT) + h, 2 * np.arange(NQT) + 1 - h])
        # ship order Q0 Q1 N0 N1 Q2 Q3 N2 N3 over position-chunks of 4 tiles
        tile_order = np.concatenate([pos[4 * s:4 * s + 4] for s in SHIP])
        xt = xt[:, tile_order, :]
        xc = np.ascontiguousarray(
            xt.reshape(EC, P, NCH, CHW).transpose(2, 1, 0, 3)
        ).reshape(NCH * P, EC * CHW)
        in_maps.append({"xkv": xc, "consts": consts[h]})
    return in_maps


def _unshard(results):
    out = np.zeros((B, T, D), np.float32)
    for c, om in enumerate(results):
        b, h = c // 2, c % 2
        o = np.asarray(om["out"], np.float32)               # [65, 2048]
        on = (o[:D] / o[D:D + 1]).T                         # [2048, 64]
        for i in range(NQT):
            out[b, (2 * i + h) * P:(2 * i + h + 1) * P] = on[i * P:(i + 1) * P]
    return out


def kernel(x, Wq, Wk, Wv):
    from concourse import bass_utils

    if "nc" not in _CACHE:
        _CACHE["nc"] = _build_bass()
    nc = _CACHE["nc"]
    in_maps = _shard_inputs(x, Wq, Wk, Wv)
    res = bass_utils.run_bass_kernel_spmd(nc, in_maps, core_ids=list(range(8)))
    _CACHE["last_result"] = res
    return _unshard(res.results)


# revision 44
# speedup vs baseline: 1.0349x; 1.0046x over previous
"""Trainium2 8-core causal single-head attention.

Problem: x[4,4096,768] @ Wq/Wk/Wv[768,64] -> causal softmax attention -> out[4,4096,64].

Sharding: 8 cores = 4 batches x 2 query-interleave groups. Core c handles
batch b=c//2, parity h=c%2: local q-tile i (16 tiles of 128 rows) is the
global q-tile g=2i+h. Both cores of a batch compute full-context K/V
projections locally (no collectives).

Position remap (pure data, SPMD-uniform program): the host reorders the 32
kv tiles per core as [own q tiles 0..15 | other-parity tiles 0..15]. Local
q tile i then attends kv positions {0..i} (triangular mask on position i)
and {16..16+i} (parity mask on position 16+i: zeros for h=0, ones for h=1).
Masks are input data, so one compiled program serves both parities, and the
q projection reads plain contiguous chunks (no gather, no separate xq
stream: 6.3MB input instead of 9.4MB).

Host layout: xkv is chunk-major [8 chunks][128 rows][6 ec][512 cols], chunks
shipped in consume order Q0 Q1 N0 N1 Q2 Q3 N2 N3 (ship column s of kvT holds
position: s<8 -> s, 8<=s<16 -> 8+s, 16<=s<24 -> s-8, s>=24 -> s). Each chunk
is one contiguous 768KB DMA with 6KB/partition descriptors, alternating the
two hw-DGE queues (sync + scalar) in ship order. All constants ship as one
host-prepacked blob (single DMA).

On-chip: projections contract E on partitions; scores are computed as
S^T[kv_p, q_f] = kT.T @ qT so exp output PT directly feeds PV:
outT[d1, q] += v1[kv,65].T @ PT[kv, q], v1 = [v | ones] so row 64 accumulates
the softmax denominator (host divides + scatters). Exp runs on Scalar/ACT
(its only job - no DMA issue on that queue after the input phase); masks
multiply on DVE; each output 512-block drains as its accumulation closes,
the final block in two 256-col pieces to shorten the tail.
"""

import sys

sys.path.insert(0, "/opt/trn_rl_repo")

from contextlib import ExitStack

import numpy as np
import ml_dtypes

B, T, E, D = 4, 4096, 768, 64
P = 128
TQ = T // 2          # queries per core
NQT = TQ // P        # 16 local q tiles
NKV = T // P         # 32 kv tiles
EC = E // P          # 6 contraction chunks
NCH = 8              # xkv chunks of 512 cols (4 tiles each)
CHW = 512
BF16 = ml_dtypes.bfloat16
N_WARMUP = 12        # dummy matmuls covering the input-DMA landing window

# ship chunk s holds position-tiles POS_OF_CHUNK[s]
SHIP = [0, 1, 4, 5, 2, 3, 6, 7]          # ship slot -> q/n chunk id (Q0 Q1 N0 N1 Q2 Q3 N2 N3)


def shipcol(k):
    """kv position-pair k -> ship tile-column in kvT/v1."""
    if k < 8:
        return k
    if k < 16:
        return 8 + k          # positions 8..15 live in ship chunks 4,5
    if k < 24:
        return k - 8          # positions 16..23 live in ship chunks 2,3
    return k                  # positions 24..31 live in ship chunks 6,7

_CACHE = {}


def _build_bass():
    import concourse.bacc as bacc
    import concourse.mybir as mybir
    import concourse.tile as tile

    nc = bacc.Bacc("TRN2", target_bir_lowering=False)
    f32 = mybir.dt.float32
    bf16 = mybir.dt.bfloat16

    xkv_d = nc.dram_tensor("xkv", (NCH * P, EC * CHW), bf16, kind="ExternalInput")
    # constants host-prepacked in SBUF layout: one DMA, big descriptors.
    CW_Q, CW_KV, CW_M = EC * D, EC * 2 * D, P
    CTOT = CW_Q + CW_KV + 2 * CW_M + D
    const_d = nc.dram_tensor("consts", (P, CTOT), bf16, kind="ExternalInput")
    out_d = nc.dram_tensor("out", (D + 1, TQ), f32, kind="ExternalOutput")

    with ExitStack() as ctx:
        tc = ctx.enter_context(tile.TileContext(nc))
        const = ctx.enter_context(tc.tile_pool(name="const", bufs=1))
        xpool = ctx.enter_context(tc.tile_pool(name="x", bufs=1))
        spool = ctx.enter_context(tc.tile_pool(name="sb", bufs=1))
        ptpool = ctx.enter_context(tc.tile_pool(name="pt", bufs=4))
        obpool = ctx.enter_context(tc.tile_pool(name="ob", bufs=2))
        # PSUM: scores 2x[128,1024]f32 = 4 banks, proj 2x[128,512]f32-sized =
        # 2 banks, outp [65,1024]f32 = 2 banks -> 8 banks exactly. Separate
        # pools so a long-lived proj accumulator never steals a score buffer.
        psst = ctx.enter_context(tc.tile_pool(name="psst", bufs=2, space="PSUM"))
        pprj = ctx.enter_context(tc.tile_pool(name="pprj", bufs=2, space="PSUM"))
        pout = ctx.enter_context(tc.tile_pool(name="pout", bufs=1, space="PSUM"))

        # ---- PE warmup: keep TensorE busy through the input-DMA landing so
        # the HAM clock gate ramps and real matmuls start at 2.4 GHz ----
        scratch = const.tile([P, 512], bf16)
        nc.vector.memset(scratch[:], 1.0)

        def warm(n):
            for wi in range(n):
                pw = psst.tile([P, 1024], f32, tag="ss", name=f"warm{wi}")
                nc.tensor.matmul(
                    pw[:, 0:512], lhsT=scratch[:, 0:P], rhs=scratch[:],
                    start=True, stop=True,
                )
                if wi == n - 1:
                    nc.vector.tensor_copy(scratch[0:1, 0:1], pw[0:1, 0:1])

        warm(N_WARMUP)

        # ---- input DMA on the two hw-DGE queues in ship order ----
        const_t = const.tile([P, CTOT], bf16)
        nc.sync.dma_start(out=const_t[:], in_=const_d[:])
        wq_t = const_t[:, 0:CW_Q]
        wkv_t = const_t[:, CW_Q:CW_Q + CW_KV]
        mdiag_t = const_t[:, CW_Q + CW_KV:CW_Q + CW_KV + CW_M]
        mpar_t = const_t[:, CW_Q + CW_KV + CW_M:CW_Q + CW_KV + 2 * CW_M]
        ident_t = const_t[:, CW_Q + CW_KV + 2 * CW_M:CTOT]

        xkv_t = xpool.tile([P, NCH * EC * CHW], bf16)

        def dma_xkv(s, eng, c0=0, c1=EC * CHW):
            eng.dma_start(
                out=xkv_t[:, s * EC * CHW + c0: s * EC * CHW + c1],
                in_=xkv_d[s * P:(s + 1) * P, c0:c1],
            )

        for s in range(0, NCH, 2):
            dma_xkv(s, nc.scalar)
            dma_xkv(s + 1, nc.sync)

        qT_t = spool.tile([D, TQ], bf16)
        kvT_t = spool.tile([P, T], bf16)
        v1_t = spool.tile([P, NKV * (D + 1)], bf16)
        nc.vector.memset(v1_t[:], 1.0)

        def qt_step(jb, ec, box):
            # one ec-step of the q projection for qT cols [jb*512,(jb+1)*512)
            s = (0, 1, 4, 5)[jb]
            if ec == 0:
                box["ps"] = pprj.tile([P, 512], f32, tag="pj", name=f"psq{jb}")
            ps = box["ps"]
            nc.tensor.matmul(
                ps[0:D, :],
                lhsT=wq_t[:, ec * D:(ec + 1) * D],
                rhs=xkv_t[:, s * EC * CHW + ec * CHW:(s * EC + ec + 1) * CHW],
                start=(ec == 0),
                stop=(ec == EC - 1),
            )
            if ec == EC - 1:
                nc.vector.tensor_copy(qT_t[:, jb * 512:(jb + 1) * 512], ps[0:D, :])

        def kv_step(s, ec, box):
            # one ec-step of the k/v projection for ship columns s*512..
            if ec == 0:
                box["ps"] = pprj.tile([P, 512], f32, tag="pj", name=f"pskv{s}")
            ps = box["ps"]
            nc.tensor.matmul(
                ps[:, :],
                lhsT=wkv_t[:, ec * 2 * D:(ec + 1) * 2 * D],
                rhs=xkv_t[:, s * EC * CHW + ec * CHW:(s * EC + ec + 1) * CHW],
                start=(ec == 0),
                stop=(ec == EC - 1),
            )
            if ec == EC - 1:
                nc.vector.tensor_copy(kvT_t[:, s * 512:(s + 1) * 512], ps[:, :])

        def v_transpose(s):
            # transpose the 4 v-tiles of ship chunk s into v1
            pv = pprj.tile([P, 512], bf16, tag="pj", name=f"psv{s}")
            for m in range(4):
                k = 4 * s + m
                nc.tensor.transpose(
                    pv[:, m * D:(m + 1) * D],
                    in_=kvT_t[D:2 * D, k * P:(k + 1) * P],
                    identity=ident_t[D:2 * D, :],
                )
            nc.vector.tensor_copy(
                v1_t.rearrange("p (k e) -> p k e", e=D + 1)[:, 4 * s:4 * s + 4, 0:D],
                pv.rearrange("p (m e) -> p m e", e=D)[:, 0:4, :],
            )

        def qt_proj(jb):
            box = {}
            for ec in range(EC):
                qt_step(jb, ec, box)

        def kv_proj_mm(s):
            box = {}
            for ec in range(EC):
                kv_step(s, ec, box)

        # ---- fused attention pair stream with single-matmul proj fillers:
        # ACT stays gapless while the PE slots leftover projection work into
        # its per-pair slack instead of bursting it between groups ----
        fillers = []

        def add_kv(s):
            box = {}
            for ec in range(EC):
                fillers.append(lambda s=s, ec=ec, box=box: kv_step(s, ec, box))

        def add_qt(jb):
            box = {}
            for ec in range(EC):
                fillers.append(lambda jb=jb, ec=ec, box=box: qt_step(jb, ec, box))

        def add_tr(s):
            fillers.append(lambda s=s: v_transpose(s))

        add_kv(2)            # s2 = N0 (kv positions 16..19), need by slot 6
        add_tr(2)
        add_kv(3)            # s3 = N1, need by slot 8
        add_tr(3)
        add_qt(2)            # s4 = Q2, need by slot 12 (group 1 scores)
        add_qt(3)            # s5 = Q3, need by slot 12
        add_kv(4)            # s4 kv (positions 8..11), need by slot 28
        add_tr(4)
        add_kv(5)
        add_tr(5)
        add_kv(6)            # s6 = N2 (positions 24..27), need by slot 34
        add_tr(6)
        add_kv(7)
        add_tr(7)

        # slot stream: each slot is one PSUM score tile + ONE activation over
        # the packed segments [(k, cs, ce), ...]. Group 0's first four pairs
        # run their qt0-only left half first so exp starts ~4us earlier (qt1
        # waits on the second input chunk); narrow tail pairs share an
        # activation to amortize the ~300ns ACT fixed cost; group 1 leads
        # with full-width pairs whose data landed long ago.
        def seg(cq, k):
            return (k, max((k % 16) * P, cq * 1024), (cq + 1) * 1024)

        slots = [(0, [seg(0, k)]) for k in [0, 1, 2, 3]]
        slots += [(0, [seg(0, 4), seg(0, 5)]), (0, [seg(0, 6), seg(0, 7)])]
        slots += [(0, [seg(0, k)]) for k in [16, 17, 18, 19]]
        slots += [(0, [seg(0, 20), seg(0, 21)]), (0, [seg(0, 22), seg(0, 23)])]
        slots += [(1, [seg(1, k)]) for k in [0, 1, 2, 3, 4, 5, 6, 7,
                                             16, 17, 18, 19, 20, 21, 22, 23,
                                             8, 9, 10, 11]]
        slots += [(1, [seg(1, 12), seg(1, 13)]), (1, [seg(1, 14), seg(1, 15)])]
        slots += [(1, [seg(1, k)]) for k in [24, 25, 26, 27]]
        slots += [(1, [seg(1, 28), seg(1, 29)]), (1, [seg(1, 30), seg(1, 31)])]

        outp_tiles = {}
        pend = []

        def drain(outp, lo, c0, c1):
            ob = obpool.tile([D + 1, c1 - c0], f32)
            nc.vector.tensor_copy(ob[:], outp[:, c0 - lo: c1 - lo])
            nc.sync.dma_start(out=out_d[:, c0:c1], in_=ob[:])

        def flush_pv():
            cq, k, pt, po, cs, ce = pend.pop(0)
            lo = cq * 1024
            w = ce - cs
            outp = outp_tiles[cq]
            sc = shipcol(k)
            v1k = v1_t[:, sc * (D + 1):(sc + 1) * (D + 1)]
            # unmasked halves first: the mask multiply only touches
            # pt[:, po:po+128], so the later half issues without waiting on it
            for half in sorted(range(0, w, 512), reverse=True):
                hw = min(512, w - half)
                g512 = (cs + half) // 512
                nc.tensor.matmul(
                    outp[:, cs + half - lo: cs + half - lo + hw],
                    lhsT=v1k,
                    rhs=pt[:, po + half:po + half + hw],
                    start=(k == 0),
                    stop=(k == 16 + 4 * g512 + 3),
                )
            # drain blocks as their accumulation closes (the parity pair
            # 16+j of q tile j is always the last writer of its block);
            # final block goes in two 256-col pieces to shorten the tail
            if k == 16 + 4 * (2 * cq) + 3:
                drain(outp, lo, lo, lo + 512)
            elif k == 16 + 4 * (2 * cq + 1) + 3:
                if cq == 1:
                    drain(outp, lo, 1792, 2048)
                else:
                    drain(outp, lo, lo + 512, lo + 1024)
            elif cq == 1 and k == 30:
                drain(outp, lo, 1536, 1792)

        # front: projections feeding the first slots, in ship-arrival order
        qt_proj(0)            # s0
        kv_proj_mm(0)
        qt_proj(1)            # s1
        kv_proj_mm(1)
        v_transpose(0)
        v_transpose(1)

        nf = 0  # fillers consumed
        for idx, (cq, segs) in enumerate(slots):
            if cq not in outp_tiles:
                outp_tiles[cq] = pout.tile(
                    [D + 1, 1024], f32, tag="out", name=f"outp{cq}"
                )
            wtot = sum(ce - cs for _, cs, ce in segs)
            sst = psst.tile([P, 1024], f32, tag="ss", name=f"sst{idx}")
            po = 0
            offs = []
            for k, cs, ce in segs:
                offs.append(po)
                for half in range(0, ce - cs, 512):
                    hw = min(512, ce - cs - half)
                    nc.tensor.matmul(
                        sst[:, po + half:po + half + hw],
                        lhsT=kvT_t[0:D, shipcol(k) * P:(shipcol(k) + 1) * P],
                        rhs=qT_t[:, cs + half: cs + half + hw],
                        start=True,
                        stop=True,
                    )
                po += ce - cs
            pt = ptpool.tile([P, 1024], bf16)
            nc.scalar.activation(
                pt[:, 0:wtot], sst[:, 0:wtot],
                func=mybir.ActivationFunctionType.Exp, scale=0.125,
            )
            for (k, cs, ce), po in zip(segs, offs):
                if cs == (k % 16) * P:
                    m = mdiag_t if k < 16 else mpar_t
                    nc.vector.tensor_mul(
                        pt[:, po:po + P], pt[:, po:po + P], m[:]
                    )
                pend.append((cq, k, pt, po, cs, ce))
            # proj filler steps: front-loaded so group 1's prerequisites
            # (qt2/qt3) land by slot 12 and kv7 by the tail slots
            budget = 3 if idx < 4 else (2 if idx < 14 else 1)
            while budget > 0 and nf < len(fillers):
                fillers[nf]()
                nf += 1
                budget -= 1
            # scores of the next slot issue before the PV of this one: the PE
            # never sits through the Scalar-engine exp latency
            while len(pend) > max(2, len(segs)):
                flush_pv()
        while pend:
            flush_pv()
        assert nf == len(fillers), (nf, len(fillers))

    nc.compile()
    return nc


def _shard_inputs(x, Wq, Wk, Wv):
    x = np.asarray(x, np.float32)
    wq_p = np.asarray(Wq, np.float32).astype(BF16).reshape(EC, P, D).transpose(1, 0, 2).reshape(P, EC * D)
    wkv_p = np.concatenate(
        [np.asarray(Wk, np.float32), np.asarray(Wv, np.float32)], axis=1
    ).astype(BF16).reshape(EC, P, 2 * D).transpose(1, 0, 2).reshape(P, EC * 2 * D)
    ident = np.zeros((P, D), BF16)
    ident[D:2 * D, :] = np.eye(D, dtype=BF16)
    tri = (np.arange(P)[:, None] <= np.arange(P)[None, :]).astype(BF16)
    ones = np.ones((P, P), BF16)
    zeros = np.zeros((P, P), BF16)
    consts = {
        h: np.ascontiguousarray(np.concatenate(
            [wq_p, wkv_p, tri, zeros if h == 0 else ones, ident], axis=1))
        for h in (0, 1)
    }
    in_maps = []
    for c in range(8):
        b, h = c // 2, c % 2
        xT = np.ascontiguousarray(x[b].T).astype(BF16)      # [768, 4096]
        xt = xT.reshape(E, NKV, P)
        # position order: own q tiles (2i+h) first, then other parity
        pos = np.concatenate([2 * np.arange(NQT) + h, 2 * np.arange(NQT) + 1 - h])
        # ship order Q0 Q1 N0 N1 Q2 Q3 N2 N3 over position-chunks of 4 tiles
        tile_order = np.concatenate([pos[4 * s:4 * s + 4] for s in SHIP])
        xt = xt[:, tile_order, :]
        xc = np.ascontiguousarray(
            xt.reshape(EC, P, NCH, CHW).transpose(2, 1, 0, 3)
        ).reshape(NCH * P, EC * CHW)
        in_maps.append({"xkv": xc, "consts": consts[h]})
    return in_maps


def _unshard(results):
    out = np.zeros((B, T, D), np.float32)
    for c, om in enumerate(results):
        b, h = c // 2, c % 2
        o = np.asarray(om["out"], np.float32)               # [65, 2048]
        on = (o[:D] / o[D:D + 1]).T                         # [2048, 64]
        for i in range(NQT):
            out[b, (2 * i + h) * P:(2 * i + h + 1) * P] = on[i * P:(i + 1) * P]
    return out


def kernel(x, Wq, Wk, Wv):
    from concourse import bass_utils

    if "nc" not in _CACHE:
        _CACHE["nc"] = _build_bass()
    nc = _CACHE["nc"]
    in_maps = _shard_inputs(x, Wq, Wk, Wv)
    res = bass_utils.run_bass_kernel_spmd(nc, in_maps, core_ids=list(range(8)))
    _CACHE["last_result"] = res
    return _unshard(res.results)
